# revision 34
# baseline (speedup 1.0000x reference)
"""GCN+ReLU 2-layer kernel for Trainium2, 8 NeuronCores — v2.

Changes vs v1 (baseline):
  - bf16 value path: gathered rows, one-hot S, all matmuls (PSUM stays f32).
  - S one-hot matrices built in 4-block batches (one [128,SB,128] is_equal).
  - Dense/res/activation/combine ops batched over 4 dst tiles ([128,512]).
  - Layer-1 BatchNorm deferred into layer 2: the AllGather ships pre-BN
    bf16 x plus the f32 BN stats bitcast into 4 extra bf16 rows per slice,
    so layer 1 needs no separate stats AllReduce and no post-loop BN pass.
    Layer 2 folds the affine into its weights: W1s = s*W1 rows, Wr1s =
    s*Wr1, bias2 = Wr1^T t + br1, plus a rank-1 (W1^T t) (x) deg term
    added into the dense-matmul PSUM.
  - Layer-1 res branch feeds from host-transposed bf16 hsT; layer-1
    gathers from a host-provided bf16 copy of h (no loads/transposes).
  - Per-tile output stores overlap the tile loop (no store phase).
  - 8 gather buffers in flight across the 4 SWDGE queues.
"""
import sys
sys.path.insert(0, '/opt/trn_rl_repo')

from contextlib import ExitStack

import numpy as np
import ml_dtypes

import concourse.bass as bass
import concourse.bacc as bacc_mod
import concourse.mybir as mybir
from concourse import bass_utils
from concourse.tile import TileContext

P = 128
D = 128
N_CORES = 8
N_BANKS = 4
BN_EPS = 1e-5
SB = 4          # S one-hot blocks per build instruction
CT = 4          # dst tiles per dense/act/combine group
SROWS = 4       # bf16 rows used to ship the f32 [128,2] stats

F32 = mybir.dt.float32
BF16 = mybir.dt.bfloat16
I16 = mybir.dt.int16
Alu = mybir.AluOpType
Act = mybir.ActivationFunctionType

BF = ml_dtypes.bfloat16


def _preprocess(src, dst, N, n_cores):
    """Balanced tile-to-core assignment + edge grouping.

    Global 128-node dst tiles are assigned to cores such that tiles with
    similar per-bank block-count vectors share a program position — this
    minimizes the cross-core max padding in the uniform nblk profile.
    Assignment is constrained so a tile stays within its original core-PAIR
    region, which keeps the gather-bank node sets fixed (banks = windows of
    2 adjacent core slices in the permuted row layout). Layer 1's gather
    source is uploaded in the same permuted layout as the AllGather output,
    so both layers share one index table.
    """
    src = src.astype(np.int64)
    dst = dst.astype(np.int64)
    NT = -(-(N // n_cores) // P)       # tiles per core (98)
    TPP = 2 * NT                        # tiles per pair region (196)
    GT = (n_cores // 2) * TPP           # global tile slots (784)
    SLICE_E = NT * P + SROWS
    BANKE = 2 * SLICE_E

    gg_d = dst // P
    gg_s = src // P
    bank_e = gg_s // TPP                # src bank = owner pair region

    cnt = np.zeros((GT, N_BANKS), np.int64)
    np.add.at(cnt, (gg_d, bank_e), 1)
    blocks = -(-cnt // P)

    owner = np.zeros(GT, np.int64)
    pos = np.zeros(GT, np.int64)
    for p in range(n_cores // 2):
        tiles = list(range(p * TPP, (p + 1) * TPP))
        tiles.sort(key=lambda gi: tuple(blocks[gi]))
        for i in range(NT):
            owner[tiles[2 * i]] = 2 * p
            pos[tiles[2 * i]] = i
            owner[tiles[2 * i + 1]] = 2 * p + 1
            pos[tiles[2 * i + 1]] = i

    nblk = [[0] * N_BANKS for _ in range(NT)]
    for gi in range(GT):
        for b in range(N_BANKS):
            nblk[pos[gi]][b] = max(nblk[pos[gi]][b], int(blocks[gi][b]))
    assert max(max(r) for r in nblk) * P <= 1024, "gather ni limit"

    totblk = sum(sum(r) for r in nblk)
    totcols = totblk * 8

    # per-edge permuted source index (within its bank window) + local dst
    lidx_s = owner[gg_s] * SLICE_E + pos[gg_s] * P + src % P - bank_e * BANKE
    assert lidx_s.min() >= 0 and lidx_s.max() < 32768
    dloc = dst % P
    pos_d = pos[gg_d]
    owner_d = owner[gg_d]

    # rows_of_core[c][j] = global node id at slice column j (-1 = pad)
    rows_l = []
    for c in range(n_cores):
        rows = np.full(NT * P, -1, np.int64)
        for gi in np.where(owner == c)[0]:
            n0 = gi * P
            nn = min(P, max(0, N - n0))
            if nn > 0:
                rows[pos[gi] * P:pos[gi] * P + nn] = np.arange(n0, n0 + nn)
        rows_l.append(rows)

    per = [[None] * NT for _ in range(n_cores)]
    deg = np.zeros((n_cores, NT * P), np.float32)
    for c in range(n_cores):
        m = owner_d == c
        li, de, pd, be = lidx_s[m], dloc[m], pos_d[m], bank_e[m]
        np.add.at(deg[c], pd * P + de, 1.0)
        key = pd * N_BANKS + be
        o = np.argsort(key, kind="stable")
        li, de, key = li[o], de[o], key[o]
        lo = np.searchsorted(key, np.arange(NT * N_BANKS))
        hi = np.searchsorted(key, np.arange(NT * N_BANKS) + 1)
        for i in range(NT):
            per[c][i] = [(li[lo[i * N_BANKS + b]:hi[i * N_BANKS + b]],
                          de[lo[i * N_BANKS + b]:hi[i * N_BANKS + b]])
                         for b in range(N_BANKS)]

    idx_l, oh_l = [], []
    for c in range(n_cores):
        idx16 = np.zeros((P, totcols), np.int16)
        oh = np.full((P, totblk), -1.0, np.float64)
        blk0 = 0
        for t in range(NT):
            for b in range(N_BANKS):
                nb = nblk[t][b]
                if nb == 0:
                    continue
                ni = nb * P
                s_tb, d_tb = per[c][t][b]
                arr = np.zeros(ni, np.int64)
                arr[:len(s_tb)] = s_tb
                tile16 = arr.reshape(ni // 16, 16).T.astype(np.int16)
                idx16[:, blk0 * 8:blk0 * 8 + nb * 8] = np.tile(tile16, (8, 1))
                ohv = np.full(ni, -1.0, np.float64)
                ohv[:len(d_tb)] = d_tb
                oh[:, blk0:blk0 + nb] = ohv.reshape(nb, P).T
                blk0 += nb
        idx_l.append(idx16)
        oh_l.append(oh.astype(BF))

    meta = dict(NT=NT, SLICE_E=SLICE_E, BANKE=BANKE,
                totblk=totblk, totcols=totcols)
    return idx_l, oh_l, nblk, deg, rows_l, meta


def _build(N, nblk, n_cores):
    T_NODE = len(nblk)
    NPC_PAD = T_NODE * P
    SLICE_E = NPC_PAD + SROWS          # slice rows + stat rows in xbe/xge
    BANKE = 2 * SLICE_E                # gather bank = 2 adjacent core slices
    totblk = sum(sum(r) for r in nblk)
    totcols = totblk * 8
    NBMAX = max(max(r) for r in nblk)
    NGRP = -(-T_NODE // CT)
    groups = [list(range(n_cores))]

    nc = bacc_mod.Bacc(num_devices=n_cores, num_swdge_queues=4)

    hb = nc.dram_tensor("hb", [n_cores * SLICE_E, D], BF16,
                        kind="ExternalInput")
    hsTd = nc.dram_tensor("hsT", [P, NPC_PAD], BF16, kind="ExternalInput")
    i16ad = nc.dram_tensor("i16a", [P, totcols], I16, kind="ExternalInput")
    ohd = nc.dram_tensor("oh", [P, totblk], BF16, kind="ExternalInput")
    w0d = nc.dram_tensor("w0", [D, D], BF16, kind="ExternalInput")
    wr0d = nc.dram_tensor("wr0", [D, D], BF16, kind="ExternalInput")
    w1d = nc.dram_tensor("w1", [D, D], BF16, kind="ExternalInput")
    wr1d = nc.dram_tensor("wr1", [D, D], BF16, kind="ExternalInput")
    bsd = nc.dram_tensor("bs", [D, 8], F32, kind="ExternalInput")
    idnbd = nc.dram_tensor("idnb", [P, P], BF16, kind="ExternalInput")
    idnfd = nc.dram_tensor("idnf", [P, P], F32, kind="ExternalInput")
    iotad = nc.dram_tensor("iota", [P, SB * P], BF16, kind="ExternalInput")
    degd = nc.dram_tensor("deg", [1, NPC_PAD], BF16, kind="ExternalInput")
    yd = nc.dram_tensor("y", [NPC_PAD, D], F32, kind="ExternalOutput")

    xbe = nc.dram_tensor("xbe", [SLICE_E, D], BF16)
    # Local (per-core) AllGather output: the collective copies more, but
    # layer-2's random gathers then read core-local HBM instead of the
    # pair-shared region (which halves the pair's random-read bandwidth).
    xge = nc.dram_tensor("xge", [n_cores * SLICE_E, D], BF16)
    sti2 = nc.dram_tensor("sti2", [P, 2], F32)
    sto2 = nc.dram_tensor("sto2", [P, 2], F32, addr_space="Shared")

    # per-tile block lists: (bank, global block idx, offset within gather)
    blocks_of = []
    blk0 = 0
    for t in range(T_NODE):
        bl = []
        for b in range(N_BANKS):
            for j in range(nblk[t][b]):
                bl.append((b, blk0 + j, j))
            blk0 += nblk[t][b]
        blocks_of.append(bl)

    with TileContext(nc) as tc, ExitStack() as ctx:
        const = ctx.enter_context(tc.tile_pool(name="const", bufs=1))
        big = ctx.enter_context(tc.tile_pool(name="big", bufs=1))
        gpool = ctx.enter_context(tc.tile_pool(name="gp", bufs=24))
        spool = ctx.enter_context(tc.tile_pool(name="sp", bufs=12))
        apool = ctx.enter_context(tc.tile_pool(name="apl", bufs=3))
        vpool = ctx.enter_context(tc.tile_pool(name="vp", bufs=4))
        small = ctx.enter_context(tc.tile_pool(name="sm", bufs=2))
        pagg = ctx.enter_context(tc.tile_pool(name="pagg", bufs=2, space="PSUM"))
        pmm = ctx.enter_context(tc.tile_pool(name="pmm", bufs=2, space="PSUM"))
        pres = ctx.enter_context(tc.tile_pool(name="pres", bufs=2, space="PSUM"))
        pst = ctx.enter_context(tc.tile_pool(name="pst", bufs=2, space="PSUM"))

        def ct(shape, dtype, srcap=None, name=None):
            t = const.tile(shape, dtype, tag=name)
            if srcap is not None:
                nc.sync.dma_start(out=t[:], in_=srcap)
            return t

        w0_t = ct([D, D], BF16, w0d[:, :], "w0")
        wr0_t = ct([D, D], BF16, wr0d[:, :], "wr0")
        w1_t = ct([D, D], BF16, w1d[:, :], "w1")
        wr1_t = ct([D, D], BF16, wr1d[:, :], "wr1")
        w1s_t = ct([D, D], BF16, None, "w1s")
        wr1s_t = ct([D, D], BF16, None, "wr1s")
        bias_t = ct([D, 8], F32, bsd[:, :], "bs")
        identb = ct([P, P], BF16, idnbd[:, :], "idnb")
        identf = ct([P, P], F32, idnfd[:, :], "idnf")
        iota_t = ct([P, SB, P], BF16, iotad[:, :], "iota")
        oh_t = ct([P, totblk], BF16, ohd[:, :], "oh")
        # chunked index loads so the first gathers only wait on chunk 0
        i16a_t = ct([P, totcols], I16, None, "i16a")
        NCH = 8
        csz = -(-totcols // NCH)
        for k in range(NCH):
            a, b = k * csz, min(totcols, (k + 1) * csz)
            if a >= b:
                break
            nc.sync.dma_start(out=i16a_t[:, a:b], in_=i16ad[:, a:b])
        hsT = ct([P, NPC_PAD], BF16, hsTd[:, :], "hsT")
        degR = ct([1, NPC_PAD], BF16, degd[:, :], "deg")
        zerob = ct([P, P], BF16, None, "zerob")
        nc.vector.memset(zerob[:], 0.0)
        eps_t = ct([P, 1], F32, None, "eps")
        nc.vector.memset(eps_t[:], BN_EPS)
        u_row = ct([1, P], BF16, None, "urow")
        b2col = ct([P, 1], F32, None, "b2col")
        tcolb = ct([P, 1], BF16, None, "tcolb")
        scol = [ct([P, NGRP], F32, None, f"scol{i}") for i in range(2)]
        qcol = [ct([P, NGRP], F32, None, f"qcol{i}") for i in range(2)]

        xT = big.tile([P, NPC_PAD], BF16, tag="xT")

        qctr = [0]

        def layer(li, gsrc, i16_t, banke, w_eff, wr_eff, bcol, brbias, rank1,
                  store_xbe):
            for g in range(NGRP):
                tiles = list(range(g * CT, min(T_NODE, g * CT + CT)))
                ntl = len(tiles)
                gc = ntl * P
                g0 = g * CT * P
                gts = {}
                for t in tiles:
                    cum = 0
                    for b in range(N_BANKS):
                        nb = nblk[t][b]
                        if nb == 0:
                            continue
                        gt = gpool.tile([P, NBMAX, D], BF16, tag="g")
                        c0 = blocks_of[t][cum][1] * 8
                        lo = b * banke
                        hi = min(gsrc.shape[0], lo + banke)
                        nc.gpsimd.dma_gather(
                            out_ap=gt[:, 0:nb, :],
                            in_ap=gsrc[lo:hi, :],
                            idxs_ap=i16_t[:, c0:c0 + nb * 8],
                            num_idxs=nb * P,
                            num_idxs_reg=nb * P,
                            elem_size=D,
                            queue_num=qctr[0] % 4,
                        )
                        qctr[0] += 1
                        gts[(t, b)] = gt
                        cum += nb
                pa4 = pagg.tile([P, CT, P], F32, tag="pa")
                for tl, t in enumerate(tiles):
                    bl = blocks_of[t]
                    if not bl:
                        nc.tensor.matmul(pa4[:, tl, :], lhsT=zerob[:],
                                         rhs=zerob[:], start=True, stop=True)
                        continue
                    for k, (b, gidx, off) in enumerate(bl):
                        nc.tensor.matmul(pa4[:, tl, :],
                                         lhsT=gts[(t, b)][:, off, :],
                                         rhs=_s_for(gidx),
                                         start=(k == 0),
                                         stop=(k == len(bl) - 1))
                agg4 = apool.tile([P, CT, P], BF16, tag="agg")
                nc.scalar.activation(agg4[:, 0:ntl, :], pa4[:, 0:ntl, :],
                                     Act.Copy)
                pm = pmm.tile([P, CT * P], F32, tag="pm")
                nc.tensor.matmul(pm[:, 0:gc], lhsT=w_eff[:],
                                 rhs=agg4[:, 0:ntl, :],
                                 start=True, stop=not rank1)
                if rank1:
                    nc.tensor.matmul(pm[:, 0:gc], lhsT=u_row[0:1, :],
                                     rhs=degR[0:1, g0:g0 + gc],
                                     start=False, stop=True)
                newt = vpool.tile([P, CT * P], BF16, tag="newt")
                nc.scalar.activation(newt[:, 0:gc], pm[:, 0:gc], Act.Relu,
                                     bias=bias_t[:, bcol:bcol + 1])
                pr = pres.tile([P, CT * P], F32, tag="pr")
                srcT = hsT if li == 0 else xT
                nc.tensor.matmul(pr[:, 0:gc], lhsT=wr_eff[:],
                                 rhs=srcT[:, g0:g0 + gc],
                                 start=True, stop=True)
                rest = vpool.tile([P, CT * P], BF16, tag="rest")
                nc.scalar.activation(rest[:, 0:gc], pr[:, 0:gc], Act.Relu,
                                     bias=brbias)
                ov = xT[:, g0:g0 + gc]
                # pad / dummy-tile columns are exact zeros (zero biases +
                # zero hsT pads + no edges), so they contribute nothing
                # to the stats sums; accumulate every group uniformly.
                nc.vector.scalar_tensor_tensor(
                    out=ov, in0=newt[:, 0:gc], scalar=0.0,
                    in1=rest[:, 0:gc], op0=Alu.add, op1=Alu.add,
                    accum_out=scol[li][:, g:g + 1])
                sq = vpool.tile([P, CT * P], BF16, tag="sq")
                nc.scalar.activation(sq[:, 0:gc], ov, Act.Square,
                                     accum_out=qcol[li][:, g:g + 1])
                if store_xbe:
                    pt = pst.tile([P, CT, P], BF16, tag="pt")
                    for tl, t in enumerate(tiles):
                        nc.tensor.transpose(pt[:, tl, :],
                                            xT[:, t * P:(t + 1) * P],
                                            identb[:])
                    stv = vpool.tile([P, CT, P], BF16, tag="stv")
                    nc.scalar.activation(stv[:, 0:ntl, :], pt[:, 0:ntl, :],
                                         Act.Copy)
                    for tl, t in enumerate(tiles):
                        nc.sync.dma_start(out=xbe[t * P:(t + 1) * P, :],
                                          in_=stv[:, tl, :])

        # ---- S one-hot batch machinery (shared across layers) ----
        s_state = {}

        def _s_for(gidx):
            base = gidx - gidx % SB
            key = s_state.get("base")
            if key != base or s_state.get("layer") != s_state.get("want"):
                rem = min(SB, totblk - base)
                st = spool.tile([P, SB, P], BF16, tag="S")
                nc.vector.tensor_tensor(
                    out=st[:, 0:rem, :],
                    in0=oh_t[:, base:base + rem].to_broadcast([P, rem, P]),
                    in1=iota_t[:, 0:rem, :],
                    op=Alu.is_equal,
                )
                s_state["base"] = base
                s_state["tile"] = st
                s_state["layer"] = s_state.get("want")
            return s_state["tile"][:, gidx - s_state["base"], :]

        def stats_reduce(li):
            st_sb = small.tile([P, 2], F32, tag="stats")
            nc.vector.reduce_sum(out=st_sb[:, 0:1], in_=scol[li][:],
                                 axis=mybir.AxisListType.X)
            nc.vector.reduce_sum(out=st_sb[:, 1:2], in_=qcol[li][:],
                                 axis=mybir.AxisListType.X)
            return st_sb

        def stats_cols(stg):
            mean = small.tile([P, 1], F32, tag="mean")
            nc.vector.tensor_scalar_mul(mean[:], stg[:, 0:1], 1.0 / N)
            ex2 = small.tile([P, 1], F32, tag="ex2")
            nc.vector.tensor_scalar_mul(ex2[:], stg[:, 1:2], 1.0 / N)
            var = small.tile([P, 1], F32, tag="var")
            nc.vector.tensor_tensor(out=var[:], in0=mean[:], in1=mean[:],
                                    op=Alu.mult)
            nc.vector.tensor_tensor(out=var[:], in0=ex2[:], in1=var[:],
                                    op=Alu.subtract)
            sd = small.tile([P, 1], F32, tag="sd")
            nc.scalar.activation(sd[:], var[:], Act.Sqrt, bias=eps_t[:, 0:1])
            rstd = small.tile([P, 1], F32, tag="rstd")
            nc.vector.reciprocal(rstd[:], sd[:])
            return mean, rstd

        def affine_cols(mean, rstd, gcol, becol):
            sc = small.tile([P, 1], F32, tag="scale")
            nc.vector.tensor_tensor(out=sc[:], in0=bias_t[:, gcol:gcol + 1],
                                    in1=rstd[:], op=Alu.mult)
            sh = small.tile([P, 1], F32, tag="shift")
            nc.vector.tensor_tensor(out=sh[:], in0=mean[:], in1=sc[:],
                                    op=Alu.mult)
            nc.vector.tensor_tensor(out=sh[:], in0=bias_t[:, becol:becol + 1],
                                    in1=sh[:], op=Alu.subtract)
            return sc, sh

        AP = type(xbe[0:1, 0:1])

        # ======== LAYER 1 ========
        s_state["want"] = 0
        layer(0, hb, i16a_t, BANKE, w0_t, wr0_t, 0, bias_t[:, 1:2],
              False, True)

        # local stats -> transpose -> bitcast rows appended to xbe
        st1 = stats_reduce(0)
        prx = pres.tile([P, CT * P], F32, tag="pr")
        nc.tensor.transpose(prx[0:2, 0:P], st1[:, 0:2], identf[:])
        strow = small.tile([2, P], F32, tag="strow")
        nc.scalar.activation(strow[:, :], prx[0:2, 0:P], Act.Copy)
        out_ap = xbe[NPC_PAD:NPC_PAD + SROWS, :]
        out_flat = AP(out_ap.tensor, out_ap.offset, [[P * 2, 2], [1, P * 2]])
        nc.sync.dma_start(out=out_flat, in_=strow[:, :].bitcast(BF16))

        # AllGather: slices + embedded stats; doubles as the barrier
        nc.gpsimd.collective_compute(
            "AllGather", Alu.bypass, replica_groups=groups,
            ins=[xbe.ap().opt()], outs=[xge.ap().opt()])

        # extract the 8 stat blocks, sum, transpose back to [P,2]
        stt = const.tile([2, n_cores, 2 * P], BF16, tag="stt")
        src_ap = AP(xge[0:1, 0:1].tensor, NPC_PAD * D,
                    [[2 * P, 2], [SLICE_E * D, n_cores], [1, 2 * P]])
        nc.sync.dma_start(out=stt[:, :, :], in_=src_ap)
        stt32 = stt[:, :, :].bitcast(F32)         # [2, n_cores, P]
        sacc = small.tile([2, P], F32, tag="sacc")
        nc.vector.tensor_tensor(out=sacc[:, :], in0=stt32[:, 0, :],
                                in1=stt32[:, 1, :], op=Alu.add)
        for k in range(2, n_cores):
            nc.vector.tensor_tensor(out=sacc[:, :], in0=sacc[:, :],
                                    in1=stt32[:, k, :], op=Alu.add)
        prx2 = pres.tile([P, CT * P], F32, tag="pr")
        nc.tensor.transpose(prx2[0:P, 0:2], sacc[:, 0:P], identf[0:2, 0:2])
        stg1 = small.tile([P, 2], F32, tag="stg")
        nc.scalar.activation(stg1[:, :], prx2[0:P, 0:2], Act.Copy)

        # fold BN-1 into layer-2 weights
        mean1, rstd1 = stats_cols(stg1)
        s1, t1 = affine_cols(mean1, rstd1, 2, 3)
        nc.vector.tensor_scalar(out=w1s_t[:], in0=w1_t[:],
                                scalar1=s1[:, 0:1], scalar2=None,
                                op0=Alu.mult)
        nc.vector.tensor_scalar(out=wr1s_t[:], in0=wr1_t[:],
                                scalar1=s1[:, 0:1], scalar2=None,
                                op0=Alu.mult)
        nc.vector.tensor_copy(tcolb[:], t1[:])
        pm0 = pmm.tile([P, CT * P], F32, tag="pm")
        nc.tensor.matmul(pm0[:, 0:1], lhsT=w1_t[:], rhs=tcolb[:, 0:1],
                         start=True, stop=True)
        nc.tensor.matmul(pm0[:, 1:2], lhsT=wr1_t[:], rhs=tcolb[:, 0:1],
                         start=True, stop=True)
        ucol = small.tile([P, 1], BF16, tag="ucol")
        nc.scalar.activation(ucol[:], pm0[:, 0:1], Act.Copy)
        nc.scalar.activation(b2col[:], pm0[:, 1:2], Act.Copy)
        nc.vector.tensor_tensor(out=b2col[:], in0=b2col[:],
                                in1=bias_t[:, 5:6], op=Alu.add)
        pt3 = pst.tile([P, CT, P], BF16, tag="pt")
        nc.tensor.transpose(pt3[0:1, 0, :], ucol[:, 0:1], identb[:])
        nc.scalar.activation(u_row[:, :], pt3[0:1, 0, :], Act.Copy)

        # ======== LAYER 2 ========
        s_state["want"] = 1
        s_state["base"] = None
        layer(1, xge, i16a_t, BANKE, w1s_t, wr1s_t, 4, b2col[:, 0:1],
              True, False)

        # layer-2 stats AllReduce + final BN apply + store y
        st2 = stats_reduce(1)
        nc.sync.dma_start(out=sti2[:, :], in_=st2[:])
        nc.gpsimd.collective_compute(
            "AllReduce", Alu.add, replica_groups=groups,
            ins=[sti2.ap().opt()], outs=[sto2.ap().opt()])
        stg2 = small.tile([P, 2], F32, tag="stg")
        nc.sync.dma_start(out=stg2[:], in_=sto2[:, :])
        mean2, rstd2 = stats_cols(stg2)
        sc2, sh2 = affine_cols(mean2, rstd2, 6, 7)
        for g in range(NGRP):
            tiles = list(range(g * CT, min(T_NODE, g * CT + CT)))
            ntl = len(tiles)
            gc = ntl * P
            g0 = g * CT * P
            stage = vpool.tile([P, CT * P], F32, tag="ystg")
            nc.vector.tensor_scalar(out=stage[:, 0:gc], in0=xT[:, g0:g0 + gc],
                                    scalar1=sc2[:, 0:1], scalar2=sh2[:, 0:1],
                                    op0=Alu.mult, op1=Alu.add)
            prf = pres.tile([P, CT * P], F32, tag="pr")
            for tl, t in enumerate(tiles):
                nc.tensor.transpose(prf[:, tl * P:(tl + 1) * P],
                                    stage[:, tl * P:(tl + 1) * P],
                                    identf[:])
            ystv = vpool.tile([P, CT * P], F32, tag="ystv")
            nc.scalar.activation(ystv[:, 0:gc], prf[:, 0:gc], Act.Copy)
            for tl, t in enumerate(tiles):
                nc.sync.dma_start(out=yd[t * P:(t + 1) * P, :],
                                  in_=ystv[:, tl * P:(tl + 1) * P])
    nc.compile()
    return nc


def _make_in_maps(inputs, pre, n_cores):
    idx_l, oh_l, nblk, deg, rows_l, meta = pre
    h = np.asarray(inputs["h"], np.float32)
    NT = meta["NT"]
    SLICE_E = meta["SLICE_E"]
    NPC_PAD = NT * P

    bs = np.stack([
        np.asarray(inputs["b0"], np.float32),
        np.asarray(inputs["br0"], np.float32),
        np.asarray(inputs["g0"], np.float32),
        np.asarray(inputs["be0"], np.float32),
        np.asarray(inputs["b1"], np.float32),
        np.asarray(inputs["br1"], np.float32),
        np.asarray(inputs["g1"], np.float32),
        np.asarray(inputs["be1"], np.float32),
    ], axis=1)
    iota = np.tile(np.arange(P, dtype=np.float32), (P, SB)).astype(BF)

    # permuted gather source: same row layout as the AllGather output
    hb_perm = np.zeros((n_cores * SLICE_E, D), np.float32)
    for c in range(n_cores):
        rows = rows_l[c]
        valid = rows >= 0
        hb_perm[c * SLICE_E:c * SLICE_E + NPC_PAD][valid] = h[rows[valid]]
    hb16 = hb_perm.astype(BF)

    in_maps = []
    for c in range(n_cores):
        rows = rows_l[c]
        valid = rows >= 0
        hsT_c = np.zeros((P, NPC_PAD), np.float32)
        hsT_c[:, valid] = h[rows[valid]].T
        in_maps.append({
            "hb": hb16,
            "hsT": hsT_c.astype(BF),
            "i16a": idx_l[c],
            "oh": oh_l[c],
            "w0": np.asarray(inputs["W0"], np.float32).astype(BF),
            "wr0": np.asarray(inputs["Wr0"], np.float32).astype(BF),
            "w1": np.asarray(inputs["W1"], np.float32).astype(BF),
            "wr1": np.asarray(inputs["Wr1"], np.float32).astype(BF),
            "bs": bs,
            "idnb": np.eye(P, dtype=np.float32).astype(BF),
            "idnf": np.eye(P, dtype=np.float32),
            "iota": iota,
            "deg": deg[c:c + 1].astype(BF),
        })
    return in_maps


def _unshard(results, pre, N, n_cores):
    _, _, _, _, rows_l, meta = pre
    out = np.zeros((N, D), np.float32)
    for c in range(n_cores):
        rows = rows_l[c]
        valid = rows >= 0
        out[rows[valid]] = results[c]["y"][valid]
    return out


def _run(inputs, n_cores=N_CORES, trace=False):
    h = np.asarray(inputs["h"], np.float32)
    src = np.asarray(inputs["src"])
    dst = np.asarray(inputs["dst"])
    N, _ = h.shape
    pre = _preprocess(src, dst, N, n_cores)
    nblk = pre[2]

    nc = _build(N, nblk, n_cores)
    in_maps = _make_in_maps(inputs, pre, n_cores)

    res = bass_utils.run_bass_kernel_spmd(
        nc, in_maps, core_ids=list(range(n_cores)), trace=trace)
    results, extra = res.results, res

    out = _unshard(results, pre, N, n_cores)
    bsz = int(inputs["batch_size"])
    return out.reshape(bsz, -1, D).astype(np.float32), extra


def kernel(**inputs):
    out, _ = _run(inputs, trace=False)
    return out


# revision 35
# speedup vs baseline: 1.0512x; 1.0512x over previous
"""GCN+ReLU 2-layer kernel for Trainium2, 8 NeuronCores — v2.

Changes vs v1 (baseline):
  - bf16 value path: gathered rows, one-hot S, all matmuls (PSUM stays f32).
  - S one-hot matrices built in 4-block batches (one [128,SB,128] is_equal).
  - Dense/res/activation/combine ops batched over 4 dst tiles ([128,512]).
  - Layer-1 BatchNorm deferred into layer 2: the AllGather ships pre-BN
    bf16 x plus the f32 BN stats bitcast into 4 extra bf16 rows per slice,
    so layer 1 needs no separate stats AllReduce and no post-loop BN pass.
    Layer 2 folds the affine into its weights: W1s = s*W1 rows, Wr1s =
    s*Wr1, bias2 = Wr1^T t + br1, plus a rank-1 (W1^T t) (x) deg term
    added into the dense-matmul PSUM.
  - Layer-1 res branch feeds from host-transposed bf16 hsT; layer-1
    gathers from a host-provided bf16 copy of h (no loads/transposes).
  - Per-tile output stores overlap the tile loop (no store phase).
  - 8 gather buffers in flight across the 4 SWDGE queues.
"""
import sys
sys.path.insert(0, '/opt/trn_rl_repo')

from contextlib import ExitStack

import numpy as np
import ml_dtypes

import concourse.bass as bass
import concourse.bacc as bacc_mod
import concourse.mybir as mybir
from concourse import bass_utils
from concourse.tile import TileContext

P = 128
D = 128
N_CORES = 8
N_BANKS = 4
BN_EPS = 1e-5
SB = 4          # S one-hot blocks per build instruction
CT = 4          # dst tiles per dense/act/combine group
SROWS = 4       # bf16 rows used to ship the f32 [128,2] stats

F32 = mybir.dt.float32
BF16 = mybir.dt.bfloat16
I16 = mybir.dt.int16
Alu = mybir.AluOpType
Act = mybir.ActivationFunctionType

BF = ml_dtypes.bfloat16


def _preprocess(src, dst, N, n_cores):
    """Balanced tile-to-core assignment + edge grouping.

    Global 128-node dst tiles are assigned to cores such that tiles with
    similar per-bank block-count vectors share a program position — this
    minimizes the cross-core max padding in the uniform nblk profile.
    Assignment is constrained so a tile stays within its original core-PAIR
    region, which keeps the gather-bank node sets fixed (banks = windows of
    2 adjacent core slices in the permuted row layout). Layer 1's gather
    source is uploaded in the same permuted layout as the AllGather output,
    so both layers share one index table.
    """
    src = src.astype(np.int64)
    dst = dst.astype(np.int64)
    NT = -(-(N // n_cores) // P)       # tiles per core (98)
    TPP = 2 * NT                        # tiles per pair region (196)
    GT = (n_cores // 2) * TPP           # global tile slots (784)
    SLICE_E = NT * P + SROWS
    BANKE = 2 * SLICE_E

    gg_d = dst // P
    gg_s = src // P
    bank_e = gg_s // TPP                # src bank = owner pair region

    cnt = np.zeros((GT, N_BANKS), np.int64)
    np.add.at(cnt, (gg_d, bank_e), 1)
    blocks = -(-cnt // P)

    owner = np.zeros(GT, np.int64)
    pos = np.zeros(GT, np.int64)
    for p in range(n_cores // 2):
        tiles = list(range(p * TPP, (p + 1) * TPP))
        tiles.sort(key=lambda gi: tuple(blocks[gi]))
        for i in range(NT):
            owner[tiles[2 * i]] = 2 * p
            pos[tiles[2 * i]] = i
            owner[tiles[2 * i + 1]] = 2 * p + 1
            pos[tiles[2 * i + 1]] = i

    nblk = [[0] * N_BANKS for _ in range(NT)]
    for gi in range(GT):
        for b in range(N_BANKS):
            nblk[pos[gi]][b] = max(nblk[pos[gi]][b], int(blocks[gi][b]))
    assert max(max(r) for r in nblk) * P <= 1024, "gather ni limit"

    totblk = sum(sum(r) for r in nblk)
    totcols = totblk * 8

    # per-edge permuted source index (within its bank window) + local dst
    lidx_s = owner[gg_s] * SLICE_E + pos[gg_s] * P + src % P - bank_e * BANKE
    assert lidx_s.min() >= 0 and lidx_s.max() < 32768
    dloc = dst % P
    pos_d = pos[gg_d]
    owner_d = owner[gg_d]

    # rows_of_core[c][j] = global node id at slice column j (-1 = pad)
    rows_l = []
    for c in range(n_cores):
        rows = np.full(NT * P, -1, np.int64)
        for gi in np.where(owner == c)[0]:
            n0 = gi * P
            nn = min(P, max(0, N - n0))
            if nn > 0:
                rows[pos[gi] * P:pos[gi] * P + nn] = np.arange(n0, n0 + nn)
        rows_l.append(rows)

    per = [[None] * NT for _ in range(n_cores)]
    deg = np.zeros((n_cores, NT * P), np.float32)
    for c in range(n_cores):
        m = owner_d == c
        li, de, pd, be = lidx_s[m], dloc[m], pos_d[m], bank_e[m]
        np.add.at(deg[c], pd * P + de, 1.0)
        key = pd * N_BANKS + be
        o = np.argsort(key, kind="stable")
        li, de, key = li[o], de[o], key[o]
        lo = np.searchsorted(key, np.arange(NT * N_BANKS))
        hi = np.searchsorted(key, np.arange(NT * N_BANKS) + 1)
        for i in range(NT):
            per[c][i] = [(li[lo[i * N_BANKS + b]:hi[i * N_BANKS + b]],
                          de[lo[i * N_BANKS + b]:hi[i * N_BANKS + b]])
                         for b in range(N_BANKS)]

    idx_l, oh_l = [], []
    for c in range(n_cores):
        idx16 = np.zeros((P, totcols), np.int16)
        oh = np.full((P, totblk), -1.0, np.float64)
        blk0 = 0
        for t in range(NT):
            for b in range(N_BANKS):
                nb = nblk[t][b]
                if nb == 0:
                    continue
                ni = nb * P
                s_tb, d_tb = per[c][t][b]
                arr = np.zeros(ni, np.int64)
                arr[:len(s_tb)] = s_tb
                tile16 = arr.reshape(ni // 16, 16).T.astype(np.int16)
                idx16[:, blk0 * 8:blk0 * 8 + nb * 8] = np.tile(tile16, (8, 1))
                ohv = np.full(ni, -1.0, np.float64)
                ohv[:len(d_tb)] = d_tb
                oh[:, blk0:blk0 + nb] = ohv.reshape(nb, P).T
                blk0 += nb
        idx_l.append(idx16)
        oh_l.append(oh.astype(BF))

    meta = dict(NT=NT, SLICE_E=SLICE_E, BANKE=BANKE,
                totblk=totblk, totcols=totcols)
    return idx_l, oh_l, nblk, deg, rows_l, meta


def _build(N, nblk, n_cores):
    T_NODE = len(nblk)
    NPC_PAD = T_NODE * P
    SLICE_E = NPC_PAD + SROWS          # slice rows + stat rows in xbe/xge
    BANKE = 2 * SLICE_E                # gather bank = 2 adjacent core slices
    totblk = sum(sum(r) for r in nblk)
    totcols = totblk * 8
    NBMAX = max(max(r) for r in nblk)
    NGRP = -(-T_NODE // CT)
    groups = [list(range(n_cores))]

    nc = bacc_mod.Bacc(num_devices=n_cores, num_swdge_queues=4)

    hb = nc.dram_tensor("hb", [n_cores * SLICE_E, D], BF16,
                        kind="ExternalInput")
    hsTd = nc.dram_tensor("hsT", [P, NPC_PAD], BF16, kind="ExternalInput")
    i16ad = nc.dram_tensor("i16a", [P, totcols], I16, kind="ExternalInput")
    ohd = nc.dram_tensor("oh", [P, totblk], BF16, kind="ExternalInput")
    w0d = nc.dram_tensor("w0", [D, D], BF16, kind="ExternalInput")
    wr0d = nc.dram_tensor("wr0", [D, D], BF16, kind="ExternalInput")
    w1d = nc.dram_tensor("w1", [D, D], BF16, kind="ExternalInput")
    wr1d = nc.dram_tensor("wr1", [D, D], BF16, kind="ExternalInput")
    bsd = nc.dram_tensor("bs", [D, 8], F32, kind="ExternalInput")
    idnbd = nc.dram_tensor("idnb", [P, P], BF16, kind="ExternalInput")
    idnfd = nc.dram_tensor("idnf", [P, P], F32, kind="ExternalInput")
    iotad = nc.dram_tensor("iota", [P, SB * P], BF16, kind="ExternalInput")
    degd = nc.dram_tensor("deg", [1, NPC_PAD], BF16, kind="ExternalInput")
    yd = nc.dram_tensor("y", [NPC_PAD, D], F32, kind="ExternalOutput")

    xbe = nc.dram_tensor("xbe", [SLICE_E, D], BF16)
    # Local (per-core) AllGather output: the collective copies more, but
    # layer-2's random gathers then read core-local HBM instead of the
    # pair-shared region (which halves the pair's random-read bandwidth).
    xge = nc.dram_tensor("xge", [n_cores * SLICE_E, D], BF16)
    sti2 = nc.dram_tensor("sti2", [P, 2], F32)
    sto2 = nc.dram_tensor("sto2", [P, 2], F32, addr_space="Shared")

    # per-tile block lists: (bank, global block idx, offset within gather)
    blocks_of = []
    blk0 = 0
    for t in range(T_NODE):
        bl = []
        for b in range(N_BANKS):
            for j in range(nblk[t][b]):
                bl.append((b, blk0 + j, j))
            blk0 += nblk[t][b]
        blocks_of.append(bl)

    with TileContext(nc) as tc, ExitStack() as ctx:
        const = ctx.enter_context(tc.tile_pool(name="const", bufs=1))
        big = ctx.enter_context(tc.tile_pool(name="big", bufs=1))
        gpool = ctx.enter_context(tc.tile_pool(name="gp", bufs=16))
        spool = ctx.enter_context(tc.tile_pool(name="sp", bufs=8))
        apool = ctx.enter_context(tc.tile_pool(name="apl", bufs=2))
        vpool = ctx.enter_context(tc.tile_pool(name="vp", bufs=3))
        small = ctx.enter_context(tc.tile_pool(name="sm", bufs=2))
        pagg = ctx.enter_context(tc.tile_pool(name="pagg", bufs=2, space="PSUM"))
        pmm = ctx.enter_context(tc.tile_pool(name="pmm", bufs=2, space="PSUM"))
        pres = ctx.enter_context(tc.tile_pool(name="pres", bufs=2, space="PSUM"))
        pst = ctx.enter_context(tc.tile_pool(name="pst", bufs=2, space="PSUM"))

        def ct(shape, dtype, srcap=None, name=None):
            t = const.tile(shape, dtype, tag=name)
            if srcap is not None:
                nc.sync.dma_start(out=t[:], in_=srcap)
            return t

        w0_t = ct([D, D], BF16, w0d[:, :], "w0")
        wr0_t = ct([D, D], BF16, wr0d[:, :], "wr0")
        w1_t = ct([D, D], BF16, w1d[:, :], "w1")
        wr1_t = ct([D, D], BF16, wr1d[:, :], "wr1")
        w1s_t = ct([D, D], BF16, None, "w1s")
        wr1s_t = ct([D, D], BF16, None, "wr1s")
        bias_t = ct([D, 8], F32, bsd[:, :], "bs")
        identb = ct([P, P], BF16, idnbd[:, :], "idnb")
        identf = ct([P, P], F32, idnfd[:, :], "idnf")
        iota_t = ct([P, SB, P], BF16, iotad[:, :], "iota")
        oh_t = ct([P, totblk], BF16, ohd[:, :], "oh")
        # chunked index loads so the first gathers only wait on chunk 0
        i16a_t = ct([P, totcols], I16, None, "i16a")
        NCH = 8
        csz = -(-totcols // NCH)
        for k in range(NCH):
            a, b = k * csz, min(totcols, (k + 1) * csz)
            if a >= b:
                break
            nc.sync.dma_start(out=i16a_t[:, a:b], in_=i16ad[:, a:b])
        hsT = ct([P, NPC_PAD], BF16, hsTd[:, :], "hsT")
        degR = ct([1, NPC_PAD], BF16, degd[:, :], "deg")
        zerob = ct([P, P], BF16, None, "zerob")
        nc.vector.memset(zerob[:], 0.0)
        eps_t = ct([P, 1], F32, None, "eps")
        nc.vector.memset(eps_t[:], BN_EPS)
        u_row = ct([1, P], BF16, None, "urow")
        b2col = ct([P, 1], F32, None, "b2col")
        tcolb = ct([P, 1], BF16, None, "tcolb")
        scol = [ct([P, NGRP], F32, None, f"scol{i}") for i in range(2)]
        qcol = [ct([P, NGRP], F32, None, f"qcol{i}") for i in range(2)]

        xT = big.tile([P, NPC_PAD], BF16, tag="xT")

        qctr = [0]

        def layer(li, gsrc, i16_t, banke, w_eff, wr_eff, bcol, brbias, rank1,
                  store_xbe):
            for g in range(NGRP):
                tiles = list(range(g * CT, min(T_NODE, g * CT + CT)))
                ntl = len(tiles)
                gc = ntl * P
                g0 = g * CT * P
                gts = {}
                for t in tiles:
                    cum = 0
                    for b in range(N_BANKS):
                        nb = nblk[t][b]
                        if nb == 0:
                            continue
                        gt = gpool.tile([P, NBMAX, D], BF16, tag="g")
                        c0 = blocks_of[t][cum][1] * 8
                        lo = b * banke
                        hi = min(gsrc.shape[0], lo + banke)
                        nc.gpsimd.dma_gather(
                            out_ap=gt[:, 0:nb, :],
                            in_ap=gsrc[lo:hi, :],
                            idxs_ap=i16_t[:, c0:c0 + nb * 8],
                            num_idxs=nb * P,
                            num_idxs_reg=nb * P,
                            elem_size=D,
                            queue_num=qctr[0] % 4,
                        )
                        qctr[0] += 1
                        gts[(t, b)] = gt
                        cum += nb
                pa4 = pagg.tile([P, CT, P], F32, tag="pa")
                for tl, t in enumerate(tiles):
                    bl = blocks_of[t]
                    if not bl:
                        nc.tensor.matmul(pa4[:, tl, :], lhsT=zerob[:],
                                         rhs=zerob[:], start=True, stop=True)
                        continue
                    for k, (b, gidx, off) in enumerate(bl):
                        nc.tensor.matmul(pa4[:, tl, :],
                                         lhsT=gts[(t, b)][:, off, :],
                                         rhs=_s_for(gidx),
                                         start=(k == 0),
                                         stop=(k == len(bl) - 1))
                agg4 = apool.tile([P, CT, P], BF16, tag="agg")
                nc.scalar.activation(agg4[:, 0:ntl, :], pa4[:, 0:ntl, :],
                                     Act.Copy)
                pm = pmm.tile([P, CT * P], F32, tag="pm")
                nc.tensor.matmul(pm[:, 0:gc], lhsT=w_eff[:],
                                 rhs=agg4[:, 0:ntl, :],
                                 start=True, stop=not rank1)
                if rank1:
                    nc.tensor.matmul(pm[:, 0:gc], lhsT=u_row[0:1, :],
                                     rhs=degR[0:1, g0:g0 + gc],
                                     start=False, stop=True)
                newt = vpool.tile([P, CT * P], BF16, tag="newt")
                nc.scalar.activation(newt[:, 0:gc], pm[:, 0:gc], Act.Relu,
                                     bias=bias_t[:, bcol:bcol + 1])
                pr = pres.tile([P, CT * P], F32, tag="pr")
                srcT = hsT if li == 0 else xT
                nc.tensor.matmul(pr[:, 0:gc], lhsT=wr_eff[:],
                                 rhs=srcT[:, g0:g0 + gc],
                                 start=True, stop=True)
                rest = vpool.tile([P, CT * P], BF16, tag="rest")
                nc.scalar.activation(rest[:, 0:gc], pr[:, 0:gc], Act.Relu,
                                     bias=brbias)
                ov = xT[:, g0:g0 + gc]
                # pad / dummy-tile columns are exact zeros (zero biases +
                # zero hsT pads + no edges), so they contribute nothing
                # to the stats sums; accumulate every group uniformly.
                nc.vector.scalar_tensor_tensor(
                    out=ov, in0=newt[:, 0:gc], scalar=0.0,
                    in1=rest[:, 0:gc], op0=Alu.add, op1=Alu.add,
                    accum_out=scol[li][:, g:g + 1])
                sq = vpool.tile([P, CT * P], BF16, tag="sq")
                nc.scalar.activation(sq[:, 0:gc], ov, Act.Square,
                                     accum_out=qcol[li][:, g:g + 1])
                if store_xbe:
                    pt = pst.tile([P, CT, P], BF16, tag="pt")
                    for tl, t in enumerate(tiles):
                        nc.tensor.transpose(pt[:, tl, :],
                                            xT[:, t * P:(t + 1) * P],
                                            identb[:])
                    stv = vpool.tile([P, CT, P], BF16, tag="stv")
                    nc.scalar.activation(stv[:, 0:ntl, :], pt[:, 0:ntl, :],
                                         Act.Copy)
                    for tl, t in enumerate(tiles):
                        nc.sync.dma_start(out=xbe[t * P:(t + 1) * P, :],
                                          in_=stv[:, tl, :])

        # ---- S one-hot batch machinery (shared across layers) ----
        s_state = {}

        def _s_for(gidx):
            base = gidx - gidx % SB
            key = s_state.get("base")
            if key != base or s_state.get("layer") != s_state.get("want"):
                rem = min(SB, totblk - base)
                st = spool.tile([P, SB, P], BF16, tag="S")
                nc.vector.tensor_tensor(
                    out=st[:, 0:rem, :],
                    in0=oh_t[:, base:base + rem].to_broadcast([P, rem, P]),
                    in1=iota_t[:, 0:rem, :],
                    op=Alu.is_equal,
                )
                s_state["base"] = base
                s_state["tile"] = st
                s_state["layer"] = s_state.get("want")
            return s_state["tile"][:, gidx - s_state["base"], :]

        def stats_reduce(li):
            st_sb = small.tile([P, 2], F32, tag="stats")
            nc.vector.reduce_sum(out=st_sb[:, 0:1], in_=scol[li][:],
                                 axis=mybir.AxisListType.X)
            nc.vector.reduce_sum(out=st_sb[:, 1:2], in_=qcol[li][:],
                                 axis=mybir.AxisListType.X)
            return st_sb

        def stats_cols(stg):
            mean = small.tile([P, 1], F32, tag="mean")
            nc.vector.tensor_scalar_mul(mean[:], stg[:, 0:1], 1.0 / N)
            ex2 = small.tile([P, 1], F32, tag="ex2")
            nc.vector.tensor_scalar_mul(ex2[:], stg[:, 1:2], 1.0 / N)
            var = small.tile([P, 1], F32, tag="var")
            nc.vector.tensor_tensor(out=var[:], in0=mean[:], in1=mean[:],
                                    op=Alu.mult)
            nc.vector.tensor_tensor(out=var[:], in0=ex2[:], in1=var[:],
                                    op=Alu.subtract)
            sd = small.tile([P, 1], F32, tag="sd")
            nc.scalar.activation(sd[:], var[:], Act.Sqrt, bias=eps_t[:, 0:1])
            rstd = small.tile([P, 1], F32, tag="rstd")
            nc.vector.reciprocal(rstd[:], sd[:])
            return mean, rstd

        def affine_cols(mean, rstd, gcol, becol):
            sc = small.tile([P, 1], F32, tag="scale")
            nc.vector.tensor_tensor(out=sc[:], in0=bias_t[:, gcol:gcol + 1],
                                    in1=rstd[:], op=Alu.mult)
            sh = small.tile([P, 1], F32, tag="shift")
            nc.vector.tensor_tensor(out=sh[:], in0=mean[:], in1=sc[:],
                                    op=Alu.mult)
            nc.vector.tensor_tensor(out=sh[:], in0=bias_t[:, becol:becol + 1],
                                    in1=sh[:], op=Alu.subtract)
            return sc, sh

        AP = type(xbe[0:1, 0:1])

        # ======== LAYER 1 ========
        s_state["want"] = 0
        layer(0, hb, i16a_t, BANKE, w0_t, wr0_t, 0, bias_t[:, 1:2],
              False, True)

        # local stats -> transpose -> bitcast rows appended to xbe
        st1 = stats_reduce(0)
        prx = pres.tile([P, CT * P], F32, tag="pr")
        nc.tensor.transpose(prx[0:2, 0:P], st1[:, 0:2], identf[:])
        strow = small.tile([2, P], F32, tag="strow")
        nc.scalar.activation(strow[:, :], prx[0:2, 0:P], Act.Copy)
        out_ap = xbe[NPC_PAD:NPC_PAD + SROWS, :]
        out_flat = AP(out_ap.tensor, out_ap.offset, [[P * 2, 2], [1, P * 2]])
        nc.sync.dma_start(out=out_flat, in_=strow[:, :].bitcast(BF16))

        # AllGather: slices + embedded stats; doubles as the barrier
        nc.gpsimd.collective_compute(
            "AllGather", Alu.bypass, replica_groups=groups,
            ins=[xbe.ap().opt()], outs=[xge.ap().opt()])

        # extract the 8 stat blocks, sum, transpose back to [P,2]
        stt = const.tile([2, n_cores, 2 * P], BF16, tag="stt")
        src_ap = AP(xge[0:1, 0:1].tensor, NPC_PAD * D,
                    [[2 * P, 2], [SLICE_E * D, n_cores], [1, 2 * P]])
        nc.sync.dma_start(out=stt[:, :, :], in_=src_ap)
        stt32 = stt[:, :, :].bitcast(F32)         # [2, n_cores, P]
        sacc = small.tile([2, P], F32, tag="sacc")
        nc.vector.tensor_tensor(out=sacc[:, :], in0=stt32[:, 0, :],
                                in1=stt32[:, 1, :], op=Alu.add)
        for k in range(2, n_cores):
            nc.vector.tensor_tensor(out=sacc[:, :], in0=sacc[:, :],
                                    in1=stt32[:, k, :], op=Alu.add)
        prx2 = pres.tile([P, CT * P], F32, tag="pr")
        nc.tensor.transpose(prx2[0:P, 0:2], sacc[:, 0:P], identf[0:2, 0:2])
        stg1 = small.tile([P, 2], F32, tag="stg")
        nc.scalar.activation(stg1[:, :], prx2[0:P, 0:2], Act.Copy)

        # fold BN-1 into layer-2 weights
        mean1, rstd1 = stats_cols(stg1)
        s1, t1 = affine_cols(mean1, rstd1, 2, 3)
        nc.vector.tensor_scalar(out=w1s_t[:], in0=w1_t[:],
                                scalar1=s1[:, 0:1], scalar2=None,
                                op0=Alu.mult)
        nc.vector.tensor_scalar(out=wr1s_t[:], in0=wr1_t[:],
                                scalar1=s1[:, 0:1], scalar2=None,
                                op0=Alu.mult)
        nc.vector.tensor_copy(tcolb[:], t1[:])
        pm0 = pmm.tile([P, CT * P], F32, tag="pm")
        nc.tensor.matmul(pm0[:, 0:1], lhsT=w1_t[:], rhs=tcolb[:, 0:1],
                         start=True, stop=True)
        nc.tensor.matmul(pm0[:, 1:2], lhsT=wr1_t[:], rhs=tcolb[:, 0:1],
                         start=True, stop=True)
        ucol = small.tile([P, 1], BF16, tag="ucol")
        nc.scalar.activation(ucol[:], pm0[:, 0:1], Act.Copy)
        nc.scalar.activation(b2col[:], pm0[:, 1:2], Act.Copy)
        nc.vector.tensor_tensor(out=b2col[:], in0=b2col[:],
                                in1=bias_t[:, 5:6], op=Alu.add)
        pt3 = pst.tile([P, CT, P], BF16, tag="pt")
        nc.tensor.transpose(pt3[0:1, 0, :], ucol[:, 0:1], identb[:])
        nc.scalar.activation(u_row[:, :], pt3[0:1, 0, :], Act.Copy)

        # ======== LAYER 2 ========
        s_state["want"] = 1
        s_state["base"] = None
        layer(1, xge, i16a_t, BANKE, w1s_t, wr1s_t, 4, b2col[:, 0:1],
              True, False)

        # layer-2 stats AllReduce + final BN apply + store y
        st2 = stats_reduce(1)
        nc.sync.dma_start(out=sti2[:, :], in_=st2[:])
        nc.gpsimd.collective_compute(
            "AllReduce", Alu.add, replica_groups=groups,
            ins=[sti2.ap().opt()], outs=[sto2.ap().opt()])
        stg2 = small.tile([P, 2], F32, tag="stg")
        nc.sync.dma_start(out=stg2[:], in_=sto2[:, :])
        mean2, rstd2 = stats_cols(stg2)
        sc2, sh2 = affine_cols(mean2, rstd2, 6, 7)
        for g in range(NGRP):
            tiles = list(range(g * CT, min(T_NODE, g * CT + CT)))
            ntl = len(tiles)
            gc = ntl * P
            g0 = g * CT * P
            stage = vpool.tile([P, CT * P], F32, tag="ystg")
            nc.vector.tensor_scalar(out=stage[:, 0:gc], in0=xT[:, g0:g0 + gc],
                                    scalar1=sc2[:, 0:1], scalar2=sh2[:, 0:1],
                                    op0=Alu.mult, op1=Alu.add)
            prf = pres.tile([P, CT * P], F32, tag="pr")
            for tl, t in enumerate(tiles):
                nc.tensor.transpose(prf[:, tl * P:(tl + 1) * P],
                                    stage[:, tl * P:(tl + 1) * P],
                                    identf[:])
            ystv = vpool.tile([P, CT * P], F32, tag="ystv")
            nc.scalar.activation(ystv[:, 0:gc], prf[:, 0:gc], Act.Copy)
            for tl, t in enumerate(tiles):
                nc.sync.dma_start(out=yd[t * P:(t + 1) * P, :],
                                  in_=ystv[:, tl * P:(tl + 1) * P])
    nc.compile()
    return nc


def _make_in_maps(inputs, pre, n_cores):
    idx_l, oh_l, nblk, deg, rows_l, meta = pre
    h = np.asarray(inputs["h"], np.float32)
    NT = meta["NT"]
    SLICE_E = meta["SLICE_E"]
    NPC_PAD = NT * P

    bs = np.stack([
        np.asarray(inputs["b0"], np.float32),
        np.asarray(inputs["br0"], np.float32),
        np.asarray(inputs["g0"], np.float32),
        np.asarray(inputs["be0"], np.float32),
        np.asarray(inputs["b1"], np.float32),
        np.asarray(inputs["br1"], np.float32),
        np.asarray(inputs["g1"], np.float32),
        np.asarray(inputs["be1"], np.float32),
    ], axis=1)
    iota = np.tile(np.arange(P, dtype=np.float32), (P, SB)).astype(BF)

    # permuted gather source: same row layout as the AllGather output
    hb_perm = np.zeros((n_cores * SLICE_E, D), np.float32)
    for c in range(n_cores):
        rows = rows_l[c]
        valid = rows >= 0
        hb_perm[c * SLICE_E:c * SLICE_E + NPC_PAD][valid] = h[rows[valid]]
    hb16 = hb_perm.astype(BF)

    in_maps = []
    for c in range(n_cores):
        rows = rows_l[c]
        valid = rows >= 0
        hsT_c = np.zeros((P, NPC_PAD), np.float32)
        hsT_c[:, valid] = h[rows[valid]].T
        in_maps.append({
            "hb": hb16,
            "hsT": hsT_c.astype(BF),
            "i16a": idx_l[c],
            "oh": oh_l[c],
            "w0": np.asarray(inputs["W0"], np.float32).astype(BF),
            "wr0": np.asarray(inputs["Wr0"], np.float32).astype(BF),
            "w1": np.asarray(inputs["W1"], np.float32).astype(BF),
            "wr1": np.asarray(inputs["Wr1"], np.float32).astype(BF),
            "bs": bs,
            "idnb": np.eye(P, dtype=np.float32).astype(BF),
            "idnf": np.eye(P, dtype=np.float32),
            "iota": iota,
            "deg": deg[c:c + 1].astype(BF),
        })
    return in_maps


def _unshard(results, pre, N, n_cores):
    _, _, _, _, rows_l, meta = pre
    out = np.zeros((N, D), np.float32)
    for c in range(n_cores):
        rows = rows_l[c]
        valid = rows >= 0
        out[rows[valid]] = results[c]["y"][valid]
    return out


def _run(inputs, n_cores=N_CORES, trace=False):
    h = np.asarray(inputs["h"], np.float32)
    src = np.asarray(inputs["src"])
    dst = np.asarray(inputs["dst"])
    N, _ = h.shape
    pre = _preprocess(src, dst, N, n_cores)
    nblk = pre[2]

    nc = _build(N, nblk, n_cores)
    in_maps = _make_in_maps(inputs, pre, n_cores)

    res = bass_utils.run_bass_kernel_spmd(
        nc, in_maps, core_ids=list(range(n_cores)), trace=trace)
    results, extra = res.results, res

    out = _unshard(results, pre, N, n_cores)
    bsz = int(inputs["batch_size"])
    return out.reshape(bsz, -1, D).astype(np.float32), extra


def kernel(**inputs):
    out, _ = _run(inputs, trace=False)
    return out


# revision 40
# speedup vs baseline: 1.1487x; 1.0927x over previous
"""GCN+ReLU 2-layer kernel for Trainium2, 8 NeuronCores — v2.

Changes vs v1 (baseline):
  - bf16 value path: gathered rows, one-hot S, all matmuls (PSUM stays f32).
  - S one-hot matrices built in 4-block batches (one [128,SB,128] is_equal).
  - Dense/res/activation/combine ops batched over 4 dst tiles ([128,512]).
  - Layer-1 BatchNorm deferred into layer 2: the AllGather ships pre-BN
    bf16 x plus the f32 BN stats bitcast into 4 extra bf16 rows per slice,
    so layer 1 needs no separate stats AllReduce and no post-loop BN pass.
    Layer 2 folds the affine into its weights: W1s = s*W1 rows, Wr1s =
    s*Wr1, bias2 = Wr1^T t + br1, plus a rank-1 (W1^T t) (x) deg term
    added into the dense-matmul PSUM.
  - Layer-1 res branch feeds from host-transposed bf16 hsT; layer-1
    gathers from a host-provided bf16 copy of h (no loads/transposes).
  - Per-tile output stores overlap the tile loop (no store phase).
  - 8 gather buffers in flight across the 4 SWDGE queues.
"""
import sys
sys.path.insert(0, '/opt/trn_rl_repo')

from contextlib import ExitStack

import numpy as np
import ml_dtypes

import concourse.bass as bass
import concourse.bacc as bacc_mod
import concourse.mybir as mybir
from concourse import bass_utils
from concourse.tile import TileContext

P = 128
D = 128
N_CORES = 8
N_BANKS = 4
BN_EPS = 1e-5
SB = 4          # S one-hot blocks per build instruction
CT = 4          # dst tiles per dense/act/combine group
SROWS = 4       # bf16 rows used to ship the f32 [128,2] stats

F32 = mybir.dt.float32
BF16 = mybir.dt.bfloat16
I16 = mybir.dt.int16
Alu = mybir.AluOpType
Act = mybir.ActivationFunctionType

BF = ml_dtypes.bfloat16
GMAX = 7        # max 128-slot blocks per dma_gather (ni<=896, ring-safe)


def _gather_plan(nblk):
    """Pack each bank's blocks (tiles in order, crossing tile boundaries)
    into gathers of <= GMAX blocks. Returns (plan, gmap, colbase):
    plan[gid] = (bank, [(tile, j), ...]); gmap[(t,b,j)] = (gid, offset);
    colbase[gid] = cumulative block count before gid (idx16 column base)."""
    NT = len(nblk)
    plan, gmap = [], {}
    cur, curb = [], None
    for b in range(N_BANKS):
        cur = []
        for t in range(NT):
            for j in range(nblk[t][b]):
                if len(cur) == GMAX:
                    plan.append((b, cur))
                    cur = []
                gmap[(t, b, j)] = (len(plan), len(cur))
                cur.append((t, j))
        if cur:
            plan.append((b, cur))
    colbase = []
    acc = 0
    for b, bl in plan:
        colbase.append(acc)
        acc += len(bl)
    return plan, gmap, colbase


def _preprocess(src, dst, N, n_cores):
    """Balanced tile-to-core assignment + edge grouping.

    Global 128-node dst tiles are assigned to cores such that tiles with
    similar per-bank block-count vectors share a program position — this
    minimizes the cross-core max padding in the uniform nblk profile.
    Assignment is constrained so a tile stays within its original core-PAIR
    region, which keeps the gather-bank node sets fixed (banks = windows of
    2 adjacent core slices in the permuted row layout). Layer 1's gather
    source is uploaded in the same permuted layout as the AllGather output,
    so both layers share one index table.
    """
    src = src.astype(np.int64)
    dst = dst.astype(np.int64)
    NT = -(-(N // n_cores) // P)       # tiles per core (98)
    TPP = 2 * NT                        # tiles per pair region (196)
    GT = (n_cores // 2) * TPP           # global tile slots (784)
    SLICE_E = NT * P + SROWS
    BANKE = 2 * SLICE_E

    gg_d = dst // P
    gg_s = src // P
    bank_e = gg_s // TPP                # src bank = owner pair region

    cnt = np.zeros((GT, N_BANKS), np.int64)
    np.add.at(cnt, (gg_d, bank_e), 1)
    blocks = -(-cnt // P)

    owner = np.zeros(GT, np.int64)
    pos = np.zeros(GT, np.int64)
    for p in range(n_cores // 2):
        tiles = list(range(p * TPP, (p + 1) * TPP))
        tiles.sort(key=lambda gi: tuple(blocks[gi]))
        for i in range(NT):
            owner[tiles[2 * i]] = 2 * p
            pos[tiles[2 * i]] = i
            owner[tiles[2 * i + 1]] = 2 * p + 1
            pos[tiles[2 * i + 1]] = i

    nblk = [[0] * N_BANKS for _ in range(NT)]
    for gi in range(GT):
        for b in range(N_BANKS):
            nblk[pos[gi]][b] = max(nblk[pos[gi]][b], int(blocks[gi][b]))
    assert max(max(r) for r in nblk) * P <= 1024, "gather ni limit"

    totblk = sum(sum(r) for r in nblk)
    totcols = totblk * 8

    # per-edge permuted source index (within its bank window) + local dst
    lidx_s = owner[gg_s] * SLICE_E + pos[gg_s] * P + src % P - bank_e * BANKE
    assert lidx_s.min() >= 0 and lidx_s.max() < 32768
    dloc = dst % P
    pos_d = pos[gg_d]
    owner_d = owner[gg_d]

    # rows_of_core[c][j] = global node id at slice column j (-1 = pad)
    rows_l = []
    for c in range(n_cores):
        rows = np.full(NT * P, -1, np.int64)
        for gi in np.where(owner == c)[0]:
            n0 = gi * P
            nn = min(P, max(0, N - n0))
            if nn > 0:
                rows[pos[gi] * P:pos[gi] * P + nn] = np.arange(n0, n0 + nn)
        rows_l.append(rows)

    per = [[None] * NT for _ in range(n_cores)]
    deg = np.zeros((n_cores, NT * P), np.float32)
    for c in range(n_cores):
        m = owner_d == c
        li, de, pd, be = lidx_s[m], dloc[m], pos_d[m], bank_e[m]
        np.add.at(deg[c], pd * P + de, 1.0)
        key = pd * N_BANKS + be
        o = np.argsort(key, kind="stable")
        li, de, key = li[o], de[o], key[o]
        lo = np.searchsorted(key, np.arange(NT * N_BANKS))
        hi = np.searchsorted(key, np.arange(NT * N_BANKS) + 1)
        for i in range(NT):
            per[c][i] = [(li[lo[i * N_BANKS + b]:hi[i * N_BANKS + b]],
                          de[lo[i * N_BANKS + b]:hi[i * N_BANKS + b]])
                         for b in range(N_BANKS)]

    plan, gmap, colbase = _gather_plan(nblk)

    idx_l, oh_l = [], []
    for c in range(n_cores):
        # idx16 columns in gather-plan (bank-major) order
        idx16 = np.zeros((P, totcols), np.int16)
        for gid, (b, bl) in enumerate(plan):
            for k, (t, j) in enumerate(bl):
                s_tb = per[c][t][b][0]
                arr = np.zeros(P, np.int64)
                seg = s_tb[j * P:(j + 1) * P]
                arr[:len(seg)] = seg
                tile16 = arr.reshape(8, 16).T.astype(np.int16)
                col = (colbase[gid] + k) * 8
                idx16[:, col:col + 8] = np.tile(tile16, (8, 1))
        # oh columns stay in compute (t; b; j) order
        oh = np.full((P, totblk), -1.0, np.float64)
        blk0 = 0
        for t in range(NT):
            for b in range(N_BANKS):
                nb = nblk[t][b]
                if nb == 0:
                    continue
                ni = nb * P
                d_tb = per[c][t][b][1]
                ohv = np.full(ni, -1.0, np.float64)
                ohv[:len(d_tb)] = d_tb
                oh[:, blk0:blk0 + nb] = ohv.reshape(nb, P).T
                blk0 += nb
        idx_l.append(idx16)
        oh_l.append(oh.astype(BF))

    meta = dict(NT=NT, SLICE_E=SLICE_E, BANKE=BANKE,
                totblk=totblk, totcols=totcols)
    return idx_l, oh_l, nblk, deg, rows_l, meta


def _build(N, nblk, n_cores):
    T_NODE = len(nblk)
    NPC_PAD = T_NODE * P
    SLICE_E = NPC_PAD + SROWS          # slice rows + stat rows in xbe/xge
    BANKE = 2 * SLICE_E                # gather bank = 2 adjacent core slices
    totblk = sum(sum(r) for r in nblk)
    totcols = totblk * 8
    NBMAX = max(max(r) for r in nblk)
    NGRP = -(-T_NODE // CT)
    groups = [list(range(n_cores))]

    nc = bacc_mod.Bacc(num_devices=n_cores, num_swdge_queues=4)

    hb = nc.dram_tensor("hb", [n_cores * SLICE_E, D], BF16,
                        kind="ExternalInput")
    hsTd = nc.dram_tensor("hsT", [P, NPC_PAD], BF16, kind="ExternalInput")
    i16ad = nc.dram_tensor("i16a", [P, totcols], I16, kind="ExternalInput")
    ohd = nc.dram_tensor("oh", [P, totblk], BF16, kind="ExternalInput")
    w0d = nc.dram_tensor("w0", [D, D], BF16, kind="ExternalInput")
    wr0d = nc.dram_tensor("wr0", [D, D], BF16, kind="ExternalInput")
    w1d = nc.dram_tensor("w1", [D, D], BF16, kind="ExternalInput")
    wr1d = nc.dram_tensor("wr1", [D, D], BF16, kind="ExternalInput")
    bsd = nc.dram_tensor("bs", [D, 8], F32, kind="ExternalInput")
    idnbd = nc.dram_tensor("idnb", [P, P], BF16, kind="ExternalInput")
    idnfd = nc.dram_tensor("idnf", [P, P], F32, kind="ExternalInput")
    iotad = nc.dram_tensor("iota", [P, SB * P], BF16, kind="ExternalInput")
    degd = nc.dram_tensor("deg", [1, NPC_PAD], BF16, kind="ExternalInput")
    yd = nc.dram_tensor("y", [NPC_PAD, D], F32, kind="ExternalOutput")

    xbe = nc.dram_tensor("xbe", [SLICE_E, D], BF16)
    # Local (per-core) AllGather output: the collective copies more, but
    # layer-2's random gathers then read core-local HBM instead of the
    # pair-shared region (which halves the pair's random-read bandwidth).
    xge = nc.dram_tensor("xge", [n_cores * SLICE_E, D], BF16)
    sti2 = nc.dram_tensor("sti2", [P, 2], F32)
    sto2 = nc.dram_tensor("sto2", [P, 2], F32, addr_space="Shared")

    plan, gmap, colbase = _gather_plan(nblk)
    # per-tile block lists: (gather id, offset within gather, oh column idx)
    blocks_of = []
    blk0 = 0
    for t in range(T_NODE):
        bl = []
        for b in range(N_BANKS):
            for j in range(nblk[t][b]):
                gid, off = gmap[(t, b, j)]
                bl.append((gid, off, blk0 + j))
            blk0 += nblk[t][b]
        blocks_of.append(bl)
    # gathers emitted when their first consuming tile's comp group starts
    gathers_of_group = [[] for _ in range(-(-T_NODE // CT))]
    for gid, (b, bl) in enumerate(plan):
        gathers_of_group[bl[0][0] // CT].append(gid)

    with TileContext(nc) as tc, ExitStack() as ctx:
        const = ctx.enter_context(tc.tile_pool(name="const", bufs=1))
        big = ctx.enter_context(tc.tile_pool(name="big", bufs=1))
        gpool = ctx.enter_context(tc.tile_pool(name="gp", bufs=16))
        spool = ctx.enter_context(tc.tile_pool(name="sp", bufs=8))
        apool = ctx.enter_context(tc.tile_pool(name="apl", bufs=2))
        vpool = ctx.enter_context(tc.tile_pool(name="vp", bufs=3))
        small = ctx.enter_context(tc.tile_pool(name="sm", bufs=2))
        pagg = ctx.enter_context(tc.tile_pool(name="pagg", bufs=2, space="PSUM"))
        pmm = ctx.enter_context(tc.tile_pool(name="pmm", bufs=2, space="PSUM"))
        pres = ctx.enter_context(tc.tile_pool(name="pres", bufs=2, space="PSUM"))
        pst = ctx.enter_context(tc.tile_pool(name="pst", bufs=2, space="PSUM"))

        def ct(shape, dtype, srcap=None, name=None):
            t = const.tile(shape, dtype, tag=name)
            if srcap is not None:
                nc.sync.dma_start(out=t[:], in_=srcap)
            return t

        w0_t = ct([D, D], BF16, w0d[:, :], "w0")
        wr0_t = ct([D, D], BF16, wr0d[:, :], "wr0")
        w1_t = ct([D, D], BF16, w1d[:, :], "w1")
        wr1_t = ct([D, D], BF16, wr1d[:, :], "wr1")
        w1s_t = ct([D, D], BF16, None, "w1s")
        wr1s_t = ct([D, D], BF16, None, "wr1s")
        bias_t = ct([D, 8], F32, bsd[:, :], "bs")
        identb = ct([P, P], BF16, idnbd[:, :], "idnb")
        identf = ct([P, P], F32, idnfd[:, :], "idnf")
        iota_t = ct([P, SB, P], BF16, iotad[:, :], "iota")
        oh_t = ct([P, totblk], BF16, ohd[:, :], "oh")
        # chunked index loads so the first gathers only wait on chunk 0
        i16a_t = ct([P, totcols], I16, None, "i16a")
        NCH = 8
        csz = -(-totcols // NCH)
        for k in range(NCH):
            a, b = k * csz, min(totcols, (k + 1) * csz)
            if a >= b:
                break
            nc.sync.dma_start(out=i16a_t[:, a:b], in_=i16ad[:, a:b])
        hsT = ct([P, NPC_PAD], BF16, hsTd[:, :], "hsT")
        degR = ct([1, NPC_PAD], BF16, degd[:, :], "deg")
        zerob = ct([P, P], BF16, None, "zerob")
        nc.vector.memset(zerob[:], 0.0)
        eps_t = ct([P, 1], F32, None, "eps")
        nc.vector.memset(eps_t[:], BN_EPS)
        u_row = ct([1, P], BF16, None, "urow")
        b2col = ct([P, 1], F32, None, "b2col")
        tcolb = ct([P, 1], BF16, None, "tcolb")
        scol = [ct([P, NGRP], F32, None, f"scol{i}") for i in range(2)]
        qcol = [ct([P, NGRP], F32, None, f"qcol{i}") for i in range(2)]

        xT = big.tile([P, NPC_PAD], BF16, tag="xT")

        qctr = [0]

        def layer(li, gsrc, i16_t, banke, w_eff, wr_eff, bcol, brbias, rank1,
                  store_xbe):
            gts = {}
            for g in range(NGRP):
                tiles = list(range(g * CT, min(T_NODE, g * CT + CT)))
                ntl = len(tiles)
                gc = ntl * P
                g0 = g * CT * P
                for gid in gathers_of_group[g]:
                    b, bl = plan[gid]
                    nbk = len(bl)
                    gt = gpool.tile([P, GMAX, D], BF16, tag="g")
                    c0 = colbase[gid] * 8
                    lo = b * banke
                    hi = min(gsrc.shape[0], lo + banke)
                    nc.gpsimd.dma_gather(
                        out_ap=gt[:, 0:nbk, :],
                        in_ap=gsrc[lo:hi, :],
                        idxs_ap=i16_t[:, c0:c0 + nbk * 8],
                        num_idxs=nbk * P,
                        num_idxs_reg=nbk * P,
                        elem_size=D,
                        queue_num=qctr[0] % 4,
                    )
                    qctr[0] += 1
                    gts[gid] = gt
                pa4 = pagg.tile([P, CT, P], F32, tag="pa")
                for tl, t in enumerate(tiles):
                    bl = blocks_of[t]
                    if not bl:
                        nc.tensor.matmul(pa4[:, tl, :], lhsT=zerob[:],
                                         rhs=zerob[:], start=True, stop=True)
                        continue
                    for k, (gid, off, gidx) in enumerate(bl):
                        nc.tensor.matmul(pa4[:, tl, :],
                                         lhsT=gts[gid][:, off, :],
                                         rhs=_s_for(gidx),
                                         start=(k == 0),
                                         stop=(k == len(bl) - 1))
                agg4 = apool.tile([P, CT, P], BF16, tag="agg")
                nc.scalar.activation(agg4[:, 0:ntl, :], pa4[:, 0:ntl, :],
                                     Act.Copy)
                pm = pmm.tile([P, CT * P], F32, tag="pm")
                nc.tensor.matmul(pm[:, 0:gc], lhsT=w_eff[:],
                                 rhs=agg4[:, 0:ntl, :],
                                 start=True, stop=not rank1)
                if rank1:
                    nc.tensor.matmul(pm[:, 0:gc], lhsT=u_row[0:1, :],
                                     rhs=degR[0:1, g0:g0 + gc],
                                     start=False, stop=True)
                newt = vpool.tile([P, CT * P], BF16, tag="newt")
                nc.scalar.activation(newt[:, 0:gc], pm[:, 0:gc], Act.Relu,
                                     bias=bias_t[:, bcol:bcol + 1])
                pr = pres.tile([P, CT * P], F32, tag="pr")
                srcT = hsT if li == 0 else xT
                nc.tensor.matmul(pr[:, 0:gc], lhsT=wr_eff[:],
                                 rhs=srcT[:, g0:g0 + gc],
                                 start=True, stop=True)
                rest = vpool.tile([P, CT * P], BF16, tag="rest")
                nc.scalar.activation(rest[:, 0:gc], pr[:, 0:gc], Act.Relu,
                                     bias=brbias)
                ov = xT[:, g0:g0 + gc]
                # pad / dummy-tile columns are exact zeros (zero biases +
                # zero hsT pads + no edges), so they contribute nothing
                # to the stats sums; accumulate every group uniformly.
                nc.vector.scalar_tensor_tensor(
                    out=ov, in0=newt[:, 0:gc], scalar=0.0,
                    in1=rest[:, 0:gc], op0=Alu.add, op1=Alu.add,
                    accum_out=scol[li][:, g:g + 1])
                sq = vpool.tile([P, CT * P], BF16, tag="sq")
                nc.scalar.activation(sq[:, 0:gc], ov, Act.Square,
                                     accum_out=qcol[li][:, g:g + 1])
                if store_xbe:
                    pt = pst.tile([P, CT, P], BF16, tag="pt")
                    for tl, t in enumerate(tiles):
                        nc.tensor.transpose(pt[:, tl, :],
                                            xT[:, t * P:(t + 1) * P],
                                            identb[:])
                    stv = vpool.tile([P, CT, P], BF16, tag="stv")
                    nc.scalar.activation(stv[:, 0:ntl, :], pt[:, 0:ntl, :],
                                         Act.Copy)
                    for tl, t in enumerate(tiles):
                        nc.sync.dma_start(out=xbe[t * P:(t + 1) * P, :],
                                          in_=stv[:, tl, :])

        # ---- S one-hot batch machinery (shared across layers) ----
        s_state = {}

        def _s_for(gidx):
            base = gidx - gidx % SB
            key = s_state.get("base")
            if key != base or s_state.get("layer") != s_state.get("want"):
                rem = min(SB, totblk - base)
                st = spool.tile([P, SB, P], BF16, tag="S")
                nc.vector.tensor_tensor(
                    out=st[:, 0:rem, :],
                    in0=oh_t[:, base:base + rem].to_broadcast([P, rem, P]),
                    in1=iota_t[:, 0:rem, :],
                    op=Alu.is_equal,
                )
                s_state["base"] = base
                s_state["tile"] = st
                s_state["layer"] = s_state.get("want")
            return s_state["tile"][:, gidx - s_state["base"], :]

        def stats_reduce(li):
            st_sb = small.tile([P, 2], F32, tag="stats")
            nc.vector.reduce_sum(out=st_sb[:, 0:1], in_=scol[li][:],
                                 axis=mybir.AxisListType.X)
            nc.vector.reduce_sum(out=st_sb[:, 1:2], in_=qcol[li][:],
                                 axis=mybir.AxisListType.X)
            return st_sb

        def stats_cols(stg):
            mean = small.tile([P, 1], F32, tag="mean")
            nc.vector.tensor_scalar_mul(mean[:], stg[:, 0:1], 1.0 / N)
            ex2 = small.tile([P, 1], F32, tag="ex2")
            nc.vector.tensor_scalar_mul(ex2[:], stg[:, 1:2], 1.0 / N)
            var = small.tile([P, 1], F32, tag="var")
            nc.vector.tensor_tensor(out=var[:], in0=mean[:], in1=mean[:],
                                    op=Alu.mult)
            nc.vector.tensor_tensor(out=var[:], in0=ex2[:], in1=var[:],
                                    op=Alu.subtract)
            sd = small.tile([P, 1], F32, tag="sd")
            nc.scalar.activation(sd[:], var[:], Act.Sqrt, bias=eps_t[:, 0:1])
            rstd = small.tile([P, 1], F32, tag="rstd")
            nc.vector.reciprocal(rstd[:], sd[:])
            return mean, rstd

        def affine_cols(mean, rstd, gcol, becol):
            sc = small.tile([P, 1], F32, tag="scale")
            nc.vector.tensor_tensor(out=sc[:], in0=bias_t[:, gcol:gcol + 1],
                                    in1=rstd[:], op=Alu.mult)
            sh = small.tile([P, 1], F32, tag="shift")
            nc.vector.tensor_tensor(out=sh[:], in0=mean[:], in1=sc[:],
                                    op=Alu.mult)
            nc.vector.tensor_tensor(out=sh[:], in0=bias_t[:, becol:becol + 1],
                                    in1=sh[:], op=Alu.subtract)
            return sc, sh

        AP = type(xbe[0:1, 0:1])

        # ======== LAYER 1 ========
        s_state["want"] = 0
        layer(0, hb, i16a_t, BANKE, w0_t, wr0_t, 0, bias_t[:, 1:2],
              False, True)

        # local stats -> transpose -> bitcast rows appended to xbe
        st1 = stats_reduce(0)
        prx = pres.tile([P, CT * P], F32, tag="pr")
        nc.tensor.transpose(prx[0:2, 0:P], st1[:, 0:2], identf[:])
        strow = small.tile([2, P], F32, tag="strow")
        nc.scalar.activation(strow[:, :], prx[0:2, 0:P], Act.Copy)
        out_ap = xbe[NPC_PAD:NPC_PAD + SROWS, :]
        out_flat = AP(out_ap.tensor, out_ap.offset, [[P * 2, 2], [1, P * 2]])
        nc.sync.dma_start(out=out_flat, in_=strow[:, :].bitcast(BF16))

        # AllGather: slices + embedded stats; doubles as the barrier
        nc.gpsimd.collective_compute(
            "AllGather", Alu.bypass, replica_groups=groups,
            ins=[xbe.ap().opt()], outs=[xge.ap().opt()])

        # extract the 8 stat blocks, sum, transpose back to [P,2]
        stt = const.tile([2, n_cores, 2 * P], BF16, tag="stt")
        src_ap = AP(xge[0:1, 0:1].tensor, NPC_PAD * D,
                    [[2 * P, 2], [SLICE_E * D, n_cores], [1, 2 * P]])
        nc.sync.dma_start(out=stt[:, :, :], in_=src_ap)
        stt32 = stt[:, :, :].bitcast(F32)         # [2, n_cores, P]
        sacc = small.tile([2, P], F32, tag="sacc")
        nc.vector.tensor_tensor(out=sacc[:, :], in0=stt32[:, 0, :],
                                in1=stt32[:, 1, :], op=Alu.add)
        for k in range(2, n_cores):
            nc.vector.tensor_tensor(out=sacc[:, :], in0=sacc[:, :],
                                    in1=stt32[:, k, :], op=Alu.add)
        prx2 = pres.tile([P, CT * P], F32, tag="pr")
        nc.tensor.transpose(prx2[0:P, 0:2], sacc[:, 0:P], identf[0:2, 0:2])
        stg1 = small.tile([P, 2], F32, tag="stg")
        nc.scalar.activation(stg1[:, :], prx2[0:P, 0:2], Act.Copy)

        # fold BN-1 into layer-2 weights
        mean1, rstd1 = stats_cols(stg1)
        s1, t1 = affine_cols(mean1, rstd1, 2, 3)
        nc.vector.tensor_scalar(out=w1s_t[:], in0=w1_t[:],
                                scalar1=s1[:, 0:1], scalar2=None,
                                op0=Alu.mult)
        nc.vector.tensor_scalar(out=wr1s_t[:], in0=wr1_t[:],
                                scalar1=s1[:, 0:1], scalar2=None,
                                op0=Alu.mult)
        nc.vector.tensor_copy(tcolb[:], t1[:])
        pm0 = pmm.tile([P, CT * P], F32, tag="pm")
        nc.tensor.matmul(pm0[:, 0:1], lhsT=w1_t[:], rhs=tcolb[:, 0:1],
                         start=True, stop=True)
        nc.tensor.matmul(pm0[:, 1:2], lhsT=wr1_t[:], rhs=tcolb[:, 0:1],
                         start=True, stop=True)
        ucol = small.tile([P, 1], BF16, tag="ucol")
        nc.scalar.activation(ucol[:], pm0[:, 0:1], Act.Copy)
        nc.scalar.activation(b2col[:], pm0[:, 1:2], Act.Copy)
        nc.vector.tensor_tensor(out=b2col[:], in0=b2col[:],
                                in1=bias_t[:, 5:6], op=Alu.add)
        pt3 = pst.tile([P, CT, P], BF16, tag="pt")
        nc.tensor.transpose(pt3[0:1, 0, :], ucol[:, 0:1], identb[:])
        nc.scalar.activation(u_row[:, :], pt3[0:1, 0, :], Act.Copy)

        # ======== LAYER 2 ========
        s_state["want"] = 1
        s_state["base"] = None
        layer(1, xge, i16a_t, BANKE, w1s_t, wr1s_t, 4, b2col[:, 0:1],
              True, False)

        # layer-2 stats AllReduce + final BN apply + store y
        st2 = stats_reduce(1)
        nc.sync.dma_start(out=sti2[:, :], in_=st2[:])
        nc.gpsimd.collective_compute(
            "AllReduce", Alu.add, replica_groups=groups,
            ins=[sti2.ap().opt()], outs=[sto2.ap().opt()])
        stg2 = small.tile([P, 2], F32, tag="stg")
        nc.sync.dma_start(out=stg2[:], in_=sto2[:, :])
        mean2, rstd2 = stats_cols(stg2)
        sc2, sh2 = affine_cols(mean2, rstd2, 6, 7)
        for g in range(NGRP):
            tiles = list(range(g * CT, min(T_NODE, g * CT + CT)))
            ntl = len(tiles)
            gc = ntl * P
            g0 = g * CT * P
            stage = vpool.tile([P, CT * P], F32, tag="ystg")
            nc.vector.tensor_scalar(out=stage[:, 0:gc], in0=xT[:, g0:g0 + gc],
                                    scalar1=sc2[:, 0:1], scalar2=sh2[:, 0:1],
                                    op0=Alu.mult, op1=Alu.add)
            prf = pres.tile([P, CT * P], F32, tag="pr")
            for tl, t in enumerate(tiles):
                nc.tensor.transpose(prf[:, tl * P:(tl + 1) * P],
                                    stage[:, tl * P:(tl + 1) * P],
                                    identf[:])
            ystv = vpool.tile([P, CT * P], F32, tag="ystv")
            nc.scalar.activation(ystv[:, 0:gc], prf[:, 0:gc], Act.Copy)
            for tl, t in enumerate(tiles):
                nc.sync.dma_start(out=yd[t * P:(t + 1) * P, :],
                                  in_=ystv[:, tl * P:(tl + 1) * P])
    nc.compile()
    return nc


def _make_in_maps(inputs, pre, n_cores):
    idx_l, oh_l, nblk, deg, rows_l, meta = pre
    h = np.asarray(inputs["h"], np.float32)
    NT = meta["NT"]
    SLICE_E = meta["SLICE_E"]
    NPC_PAD = NT * P

    bs = np.stack([
        np.asarray(inputs["b0"], np.float32),
        np.asarray(inputs["br0"], np.float32),
        np.asarray(inputs["g0"], np.float32),
        np.asarray(inputs["be0"], np.float32),
        np.asarray(inputs["b1"], np.float32),
        np.asarray(inputs["br1"], np.float32),
        np.asarray(inputs["g1"], np.float32),
        np.asarray(inputs["be1"], np.float32),
    ], axis=1)
    iota = np.tile(np.arange(P, dtype=np.float32), (P, SB)).astype(BF)

    # permuted gather source: same row layout as the AllGather output
    hb_perm = np.zeros((n_cores * SLICE_E, D), np.float32)
    for c in range(n_cores):
        rows = rows_l[c]
        valid = rows >= 0
        hb_perm[c * SLICE_E:c * SLICE_E + NPC_PAD][valid] = h[rows[valid]]
    hb16 = hb_perm.astype(BF)

    in_maps = []
    for c in range(n_cores):
        rows = rows_l[c]
        valid = rows >= 0
        hsT_c = np.zeros((P, NPC_PAD), np.float32)
        hsT_c[:, valid] = h[rows[valid]].T
        in_maps.append({
            "hb": hb16,
            "hsT": hsT_c.astype(BF),
            "i16a": idx_l[c],
            "oh": oh_l[c],
            "w0": np.asarray(inputs["W0"], np.float32).astype(BF),
            "wr0": np.asarray(inputs["Wr0"], np.float32).astype(BF),
            "w1": np.asarray(inputs["W1"], np.float32).astype(BF),
            "wr1": np.asarray(inputs["Wr1"], np.float32).astype(BF),
            "bs": bs,
            "idnb": np.eye(P, dtype=np.float32).astype(BF),
            "idnf": np.eye(P, dtype=np.float32),
            "iota": iota,
            "deg": deg[c:c + 1].astype(BF),
        })
    return in_maps


def _unshard(results, pre, N, n_cores):
    _, _, _, _, rows_l, meta = pre
    out = np.zeros((N, D), np.float32)
    for c in range(n_cores):
        rows = rows_l[c]
        valid = rows >= 0
        out[rows[valid]] = results[c]["y"][valid]
    return out


def _run(inputs, n_cores=N_CORES, trace=False):
    h = np.asarray(inputs["h"], np.float32)
    src = np.asarray(inputs["src"])
    dst = np.asarray(inputs["dst"])
    N, _ = h.shape
    pre = _preprocess(src, dst, N, n_cores)
    nblk = pre[2]

    nc = _build(N, nblk, n_cores)
    in_maps = _make_in_maps(inputs, pre, n_cores)

    res = bass_utils.run_bass_kernel_spmd(
        nc, in_maps, core_ids=list(range(n_cores)), trace=trace)
    results, extra = res.results, res

    out = _unshard(results, pre, N, n_cores)
    bsz = int(inputs["batch_size"])
    return out.reshape(bsz, -1, D).astype(np.float32), extra


def kernel(**inputs):
    out, _ = _run(inputs, trace=False)
    return out
